# revision 8
# baseline (speedup 1.0000x reference)
# Trainium2 Bass kernel for nn_ConLoss_798863917356 (partial-label supervised
# contrastive loss). 8-core SPMD, batch (B=2048) sharded 256 samples/core.
#
# Math (equivalent to the jax reference):
#   contrast = view-major stack of features        [N=4096, D=256]
#   adc = contrast @ contrast.T / T; mx[i] = adc[i,i] (= row max, unit vectors)
#   t[q] = argmax_c Y[q,c]*softmax(o)[q,c] = argmax over candidates of o[q,c]
#   cnt = bincount(t); Mw[p,q] = Y[p,t_q]*ps[p,t_q]/cnt[t_q]
#        = row p of (A*diag(1/max(cnt,1))) @ onehot(t)^T,  A = Y*ps
#   mean_log_prob_pos[i] = sum_q Mw[P,q]*(adc'[i,q]+adc'[i,b+q])
#                          - (mx'[i]+lse[i])*2*S[P] + Md[P]*lse[i]
#     where adc' = adc/T, mx' = diag/T, lse = ln(sum_{j!=i} exp(adc'[i,j]-mx'))
#           S[P] = sum_q Mw[P,q],  Md[P] = Mw[P,P] = A[P,t_P]/cnt[t_P]
#   loss = -mean(mean_log_prob_pos);  new_target = Y*exp(o-mxc)/rowsum(...)
#
# Collectives: one AllGather of [t_local(256) | counts_local(1024)] fp32,
# one AllGather of the per-core partial loss.
import sys

for _p in ("/opt/trn_rl_repo", "/root/.axon_site/_ro/trn_rl_repo"):
    if _p not in sys.path:
        sys.path.insert(0, _p)

import numpy as np

import concourse.bacc as bacc
import concourse.bass as bass
import concourse.mybir as mybir
import concourse.tile as tile
from concourse import bass_utils

F32 = mybir.dt.float32
F32R = mybir.dt.float32r
U32 = mybir.dt.uint32
AL = mybir.AluOpType
AF = mybir.ActivationFunctionType
AX = mybir.AxisListType

B, V, D, C = 2048, 2, 256, 1000
T = 0.07
NCORE = 8
BL = B // NCORE          # 256 local samples
RL = V * BL              # 512 local rows
N = V * B                # 4096
CP = 1024                # padded class count
BIG = 30000.0            # candidate mask offset (Y*BIG-BIG is exact for Y in {0,1})
CAT = BL + CP            # 1280 AllGather payload per core


def r(ap):
    """View an fp32 AP as float32r for fast PE matmuls."""
    return ap.bitcast(F32R)


def build_nc():
    nc = bacc.Bacc(None, target_bir_lowering=False, debug=False, num_devices=NCORE)

    f_full = nc.dram_tensor("f_full", [B, V * D], F32, kind="ExternalInput")
    f_loc = nc.dram_tensor("f_loc", [BL, V * D], F32, kind="ExternalInput")
    o_loc = nc.dram_tensor("o_loc", [BL, C], F32, kind="ExternalInput")
    y_loc = nc.dram_tensor("y_loc", [BL, C], F32, kind="ExternalInput")
    ps_loc = nc.dram_tensor("ps_loc", [BL, C], F32, kind="ExternalInput")
    iota_in = nc.dram_tensor("iota_in", [128, CP], F32, kind="ExternalInput")
    ci8_in = nc.dram_tensor("ci8_in", [128, 8], F32, kind="ExternalInput")
    ident_in = nc.dram_tensor("ident_in", [128, 128], F32, kind="ExternalInput")
    nt_out = nc.dram_tensor("nt_out", [BL, C], F32, kind="ExternalOutput")
    loss_out = nc.dram_tensor("loss_out", [1, 1], F32, kind="ExternalOutput")

    with tile.TileContext(nc) as tc:
        with (
            tc.tile_pool(name="pc", bufs=1) as pc,
            tc.tile_pool(name="pk", bufs=1) as pk,
            tc.tile_pool(name="pdram", bufs=1, space="DRAM") as pdram,
            tc.tile_pool(name="ps_tr", bufs=2, space="PSUM") as ps_tr,
            tc.tile_pool(name="ps_mm", bufs=3, space="PSUM") as ps_mm,
            tc.tile_pool(name="ps_sm", bufs=3, space="PSUM") as ps_sm,
        ):
            # ---- long-lived constants / small vectors --------------------
            iota_sb = pc.tile([128, CP], F32, name="iota_sb")
            nc.sync.dma_start(iota_sb[:], iota_in[:])
            ci8 = pc.tile([128, 8], F32, name="ci8")
            nc.sync.dma_start(ci8[:], ci8_in[:])
            ident = pc.tile([128, 128], F32, name="ident")
            nc.sync.dma_start(ident[:], ident_in[:])
            ones_col = pc.tile([128, 1], F32, name="ones_col")
            nc.vector.memset(ones_col[:], 1.0)

            fl_sb = [pc.tile([128, V * D], F32, name=f"fl{pt}") for pt in range(2)]
            for pt in range(2):
                nc.sync.dma_start(fl_sb[pt][:], f_loc[pt * 128:(pt + 1) * 128, :])

            A = [pk.tile([128, CP], F32, name=f"A{pt}") for pt in range(2)]
            Ol = [pk.tile([128, CP], F32, name=f"Ol{pt}") for pt in range(2)]
            tl_f = [pc.tile([128, 1], F32, name=f"tl{pt}") for pt in range(2)]
            diag4 = pc.tile([128, 4], F32, name="diag4")
            negdT = pc.tile([128, 4], F32, name="negdT")
            Zall = pc.tile([128, 4, 8], F32, name="Zall")
            Dall = pc.tile([128, 4, 8], F32, name="Dall")
            Dacc4 = pc.tile([128, 4], F32, name="Dacc4")
            cl_row = pc.tile([1, CP], F32, name="cl_row")

            CT = [pk.tile([128, 2, B], F32, name=f"CT{k2}") for k2 in range(2)]
            CLT = [pk.tile([128, RL], F32, name=f"CLT{k2}") for k2 in range(2)]
            AT = [pk.tile([128, BL], F32, name=f"AT{k}") for k in range(8)]
            Mw = [pk.tile([128, B], F32, name=f"Mw{pt}") for pt in range(2)]
            t_bc = pk.tile([128, B], F32, name="t_bc")

            # DRAM scratch for collectives
            cat_in = pdram.tile([CAT], F32, name="cat_in")
            cat_out = pdram.tile([NCORE, CAT], F32, name="cat_out", addr_space="Shared")
            cnt_lin = pdram.tile([CP], F32, name="cnt_lin")
            pl_in = pdram.tile([1], F32, name="pl_in")
            pl_out = pdram.tile([NCORE], F32, name="pl_out", addr_space="Shared")

            # ================= stage A: argmax / new_target / A ============
            with tc.tile_pool(name="pe1", bufs=1) as pe1:
                usum = pc.tile([128, 2], F32, name="usum")
                for pt in range(2):
                    sl = slice(pt * 128, (pt + 1) * 128)
                    o_sb = pe1.tile([128, C], F32, name=f"o_sb{pt}")
                    nc.sync.dma_start(o_sb[:], o_loc[sl, :])
                    y_sb = pe1.tile([128, C], F32, name=f"y_sb{pt}")
                    nc.sync.dma_start(y_sb[:], y_loc[sl, :])
                    p_sb = pe1.tile([128, C], F32, name=f"p_sb{pt}")
                    nc.sync.dma_start(p_sb[:], ps_loc[sl, :])

                    # masked = o + (Y-1)*BIG   (exact for candidates)
                    tmp = pe1.tile([128, C], F32, name=f"tmp{pt}")
                    nc.vector.tensor_scalar(tmp[:], y_sb[:], BIG, -BIG, AL.mult, AL.add)
                    masked = pe1.tile([128, C], F32, name=f"msk{pt}")
                    nc.vector.tensor_tensor(masked[:], tmp[:], o_sb[:], AL.add)

                    mx8 = pe1.tile([128, 8], F32, name=f"mx8{pt}")
                    idx8 = pe1.tile([128, 8], U32, name=f"idx8{pt}")
                    nc.vector.max_with_indices(mx8[:], idx8[:], masked[:])
                    nc.vector.tensor_copy(tl_f[pt][:], idx8[:, 0:1])

                    negmx = pe1.tile([128, 1], F32, name=f"negmx{pt}")
                    nc.vector.tensor_scalar_mul(negmx[:], mx8[:, 0:1], -1.0)

                    # u = exp(masked - mx)  (0 exactly for non-candidates)
                    u = pe1.tile([128, C], F32, name=f"u{pt}")
                    nc.scalar.activation(u[:], masked[:], AF.Exp,
                                         bias=negmx[:, 0:1], scale=1.0,
                                         accum_out=usum[:, pt:pt + 1])
                    rus = pe1.tile([128, 1], F32, name=f"rus{pt}")
                    nc.vector.reciprocal(rus[:], usum[:, pt:pt + 1])
                    nt_t = pe1.tile([128, C], F32, name=f"nt{pt}")
                    nc.vector.tensor_scalar(nt_t[:], u[:], rus[:, 0:1], None, AL.mult)
                    nc.sync.dma_start(nt_out[sl, :], nt_t[:])

                    # A = Y*ps (padded with zeros), Ol = onehot(t_local)
                    nc.vector.memset(A[pt][:, C:CP], 0.0)
                    nc.vector.tensor_tensor(A[pt][:, 0:C], y_sb[:], p_sb[:], AL.mult)
                    nc.vector.tensor_scalar(Ol[pt][:], iota_sb[:], tl_f[pt][:, 0:1],
                                            None, AL.is_equal)

                # local counts: cnt_loc[c] = sum_q Ol[q, c]  (PE partition-sum)
                for h in range(2):
                    cnt_ps = ps_sm.tile([1, 512], F32, name="cnt_ps", tag="sm")
                    for pt in range(2):
                        nc.tensor.matmul(cnt_ps[:],
                                         ones_col[:],
                                         Ol[pt][:, h * 512:(h + 1) * 512],
                                         start=(pt == 0), stop=(pt == 1))
                    nc.scalar.copy(cl_row[:, h * 512:(h + 1) * 512], cnt_ps[:])

                # ship [t_local | counts_local] through one AllGather
                for pt in range(2):
                    nc.sync.dma_start(cat_in[pt * 128:(pt + 1) * 128], tl_f[pt][:])
                nc.sync.dma_start(cat_in[BL:CAT], cl_row[:])
                nc.gpsimd.collective_compute(
                    "AllGather", AL.bypass,
                    replica_groups=[list(range(NCORE))],
                    ins=[cat_in[:]], outs=[cat_out[:]],
                )

                # ---- transposes: CLT (local) and CT (full), d on partitions
                for k2 in range(2):
                    psC = ps_tr.tile([128, 512], F32, name="psC", tag="tr")
                    for a in range(2):
                        for pt in range(2):
                            nc.tensor.transpose(
                                psC[:, a * 256 + pt * 128:a * 256 + pt * 128 + 128],
                                fl_sb[pt][:, (a * 2 + k2) * 128:(a * 2 + k2) * 128 + 128],
                                ident[:])
                    nc.scalar.copy(r(CLT[k2][:]), psC[:])

                for s in range(16):
                    fs = pe1.tile([128, V * D], F32, name="fs", tag="fs", bufs=3)
                    nc.sync.dma_start(fs[:], f_full[s * 128:(s + 1) * 128, :])
                    for k2 in range(2):
                        psT = ps_tr.tile([128, 512], F32, name="psT", tag="tr")
                        for c in range(2):
                            nc.tensor.transpose(
                                psT[:, c * 128:(c + 1) * 128],
                                fs[:, (c * 2 + k2) * 128:(c * 2 + k2) * 128 + 128],
                                ident[:])
                        # psT[:, c*128:...] -> CT[k2][:, c, s*128:...]
                        nc.scalar.copy(
                            r(CT[k2][:, :, s * 128:(s + 1) * 128]),
                            psT[:, 0:256].rearrange("p (c q) -> p c q", c=2))

                # ---- diag via local-block matmul (same rounding as sweep)
                for rt in range(4):
                    psD = ps_mm.tile([128, 512], F32, name="psD", tag="mm")
                    for k2 in range(2):
                        nc.tensor.matmul(psD[:],
                                         r(CLT[k2][:, rt * 128:(rt + 1) * 128]),
                                         r(CLT[k2][:]),
                                         start=(k2 == 0), stop=(k2 == 1))
                    scrI = pe1.tile([128, 128], F32, name="scrI", tag="scrI", bufs=2)
                    nc.vector.scalar_tensor_tensor(
                        scrI[:], psD[:, rt * 128:(rt + 1) * 128], 1.0, ident[:],
                        AL.mult, AL.mult, accum_out=diag4[:, rt:rt + 1])
                nc.vector.tensor_scalar_mul(negdT[:], diag4[:], -1.0 / T)

            # ================= stage B: post-AllGather =====================
            with tc.tile_pool(name="pe2", bufs=1) as pe2:
                t_row = pe2.tile([1, B], F32, name="t_row")
                nc.sync.dma_start(
                    t_row[:].rearrange("p (r x) -> p r x", r=NCORE),
                    cat_out[:, 0:BL])
                nc.gpsimd.partition_broadcast(t_bc[:], t_row[0:1, :])

                cntp = pe2.tile([128, 8, 8], F32, name="cntp")
                for rr in range(NCORE):
                    nc.sync.dma_start(
                        cntp[:, :, rr:rr + 1],
                        cat_out[rr:rr + 1, BL:CAT].rearrange("r (k p) -> p k r", p=128))
                cnt8 = pe2.tile([128, 8], F32, name="cnt8")
                with nc.allow_low_precision(reason="counts are small ints, exact in f32r"):
                    nc.vector.reduce_sum(r(cnt8[:]), cntp[:], axis=AX.X)
                c1 = pe2.tile([128, 8], F32, name="c1")
                nc.vector.tensor_scalar(c1[:], cnt8[:], 1.0, None, AL.max)
                invc8 = pe2.tile([128, 8], F32, name="invc8")
                nc.vector.reciprocal(invc8[:], c1[:])

                # cnt broadcast row for Md gather
                nc.sync.dma_start(cnt_lin[:].rearrange("(k p) -> p k", p=128), cnt8[:])
                cnt_row = pe2.tile([1, CP], F32, name="cnt_row")
                nc.sync.dma_start(cnt_row[:], cnt_lin[:])
                cnt_bc = pe2.tile([128, CP], F32, name="cnt_bc")
                nc.gpsimd.partition_broadcast(cnt_bc[:], cnt_row[0:1, :])

                # Md[p] = A[p, t_l]/cnt[t_l]
                Md = pe2.tile([128, 2], F32, name="Md")
                Ad = pe2.tile([128, 2], F32, name="Ad")
                ctl = pe2.tile([128, 2], F32, name="ctl")
                for pt in range(2):
                    scrC = pe2.tile([128, CP], F32, name="scrC", tag="scrC", bufs=2)
                    nc.vector.scalar_tensor_tensor(
                        scrC[:], A[pt][:], 1.0, Ol[pt][:],
                        AL.mult, AL.mult, accum_out=Ad[:, pt:pt + 1])
                    scrC2 = pe2.tile([128, CP], F32, name="scrC2", tag="scrC", bufs=2)
                    nc.vector.scalar_tensor_tensor(
                        scrC2[:], cnt_bc[:], 1.0, Ol[pt][:],
                        AL.mult, AL.mult, accum_out=ctl[:, pt:pt + 1])
                rctl = pe2.tile([128, 2], F32, name="rctl")
                nc.vector.reciprocal(rctl[:], ctl[:])
                nc.vector.tensor_tensor(Md[:], Ad[:], rctl[:], AL.mult)

                # O^T tiles: OT[k][c, q] = (t_q == k*128+c)
                OT = [pe2.tile([128, B], F32, name=f"OT{k}") for k in range(8)]
                for k in range(8):
                    nc.vector.tensor_scalar(r(OT[k][:]), t_bc[:], ci8[:, k:k + 1],
                                            None, AL.is_equal)

                # A'T[k] = transpose(A)[k-tile] * invcnt (scaled on evac)
                for pt in range(2):
                    for k in range(8):
                        psA = ps_tr.tile([128, 512], F32, name="psA", tag="tr")
                        nc.tensor.transpose(
                            psA[:, 0:128],
                            A[pt][:, k * 128:(k + 1) * 128], ident[:])
                        nc.scalar.activation(r(AT[k][:, pt * 128:(pt + 1) * 128]),
                                             psA[:, 0:128], AF.Copy,
                                             bias=0.0, scale=invc8[:, k:k + 1])

                # S[p] = sum_c A'T[c,p]*cnt[c]
                Ssb = pe2.tile([128, 2], F32, name="Ssb")
                for pt in range(2):
                    psS = ps_sm.tile([128, 1], F32, name="psS", tag="sm")
                    for k in range(8):
                        nc.tensor.matmul(psS[:],
                                         AT[k][:, pt * 128:(pt + 1) * 128],
                                         cnt8[:, k:k + 1],
                                         start=(k == 0), stop=(k == 7))
                    nc.scalar.copy(Ssb[:, pt:pt + 1], psS[:])

                # Mw[pt] = A'T.T @ OT   [128, 2048]
                for pt in range(2):
                    for n4 in range(4):
                        psM = ps_mm.tile([128, 512], F32, name="psM", tag="mm")
                        for k in range(8):
                            nc.tensor.matmul(psM[:],
                                             r(AT[k][:, pt * 128:(pt + 1) * 128]),
                                             r(OT[k][:, n4 * 512:(n4 + 1) * 512]),
                                             start=(k == 0), stop=(k == 7))
                        nc.scalar.copy(Mw[pt][:, n4 * 512:(n4 + 1) * 512], psM[:])

                # ---- adc sweep: Z (ACT exp accum) + D (DVE fused mul-reduce)
                for rt in range(4):
                    pt = rt % 2
                    for ct in range(8):
                        c = ct // 4
                        q4 = ct % 4
                        psmm = ps_mm.tile([128, 512], F32, name="psmm", tag="mm")
                        for k2 in range(2):
                            nc.tensor.matmul(
                                psmm[:],
                                r(CLT[k2][:, rt * 128:(rt + 1) * 128]),
                                r(CT[k2][:, c, q4 * 512:(q4 + 1) * 512]),
                                start=(k2 == 0), stop=(k2 == 1))
                        expo = pe2.tile([128, 512], F32, name="expo", tag="expo", bufs=3)
                        nc.scalar.activation(expo[:], psmm[:], AF.Exp,
                                             bias=negdT[:, rt:rt + 1], scale=1.0 / T,
                                             accum_out=Zall[:, rt, ct:ct + 1])
                        scrT = pe2.tile([128, 512], F32, name="scrT", tag="scrT", bufs=3)
                        nc.vector.scalar_tensor_tensor(
                            scrT[:], psmm[:], 1.0 / T,
                            Mw[pt][:, q4 * 512:(q4 + 1) * 512],
                            AL.mult, AL.mult,
                            accum_out=Dall[:, rt, ct:ct + 1])

                # ---- final per-row combine ------------------------------
                Zrow = pe2.tile([128, 4], F32, name="Zrow")
                nc.vector.reduce_sum(Zrow[:], Zall[:], axis=AX.X)
                nc.vector.reduce_sum(Dacc4[:], Dall[:], axis=AX.X)
                Zm = pe2.tile([128, 4], F32, name="Zm")
                nc.vector.tensor_scalar(Zm[:], Zrow[:], -1.0, None, AL.add)
                lse4 = pe2.tile([128, 4], F32, name="lse4")
                nc.scalar.activation(lse4[:], Zm[:], AF.Ln)

                S4 = pe2.tile([128, 4], F32, name="S4")
                Md4 = pe2.tile([128, 4], F32, name="Md4")
                S2x = pe2.tile([128, 2], F32, name="S2x")
                nc.vector.tensor_scalar_mul(S2x[:], Ssb[:], 2.0)
                for a in range(2):
                    nc.vector.tensor_copy(S4[:, a * 2:a * 2 + 2], S2x[:])
                    nc.vector.tensor_copy(Md4[:, a * 2:a * 2 + 2], Md[:])

                tt = pe2.tile([128, 4], F32, name="tt")
                nc.vector.tensor_tensor(tt[:], lse4[:], negdT[:], AL.subtract)
                m2 = pe2.tile([128, 4], F32, name="m2")
                nc.vector.tensor_tensor(m2[:], tt[:], S4[:], AL.mult)
                m3 = pe2.tile([128, 4], F32, name="m3")
                nc.vector.tensor_tensor(m3[:], Dacc4[:], m2[:], AL.subtract)
                m4 = pe2.tile([128, 4], F32, name="m4")
                nc.vector.tensor_tensor(m4[:], Md4[:], lse4[:], AL.mult)
                mlpp = pe2.tile([128, 4], F32, name="mlpp")
                nc.vector.tensor_tensor(mlpp[:], m3[:], m4[:], AL.add)

                psL = ps_sm.tile([1, 4], F32, name="psL", tag="sm")
                nc.tensor.matmul(psL[:], ones_col[:], mlpp[:], start=True, stop=True)
                Lrow = pe2.tile([1, 4], F32, name="Lrow")
                nc.scalar.copy(Lrow[:], psL[:])
                pl = pe2.tile([1, 1], F32, name="pl")
                nc.vector.reduce_sum(pl[:], Lrow[:], axis=AX.X)
                nc.sync.dma_start(pl_in[:], pl[:])
                nc.gpsimd.collective_compute(
                    "AllGather", AL.bypass,
                    replica_groups=[list(range(NCORE))],
                    ins=[pl_in[:]], outs=[pl_out[:]],
                )
                plr = pe2.tile([1, NCORE], F32, name="plr")
                nc.sync.dma_start(plr[:], pl_out[:])
                ptot = pe2.tile([1, 1], F32, name="ptot")
                nc.vector.reduce_sum(ptot[:], plr[:], axis=AX.X)
                lossv = pe2.tile([1, 1], F32, name="lossv")
                nc.vector.tensor_scalar_mul(lossv[:], ptot[:], -1.0 / N)
                nc.sync.dma_start(loss_out[:], lossv[:])

    nc.compile()
    return nc


_CACHE = {}


def get_nc():
    if "nc" not in _CACHE:
        _CACHE["nc"] = build_nc()
    return _CACHE["nc"]


def make_in_maps(outputs, features, Y, predicted_score):
    outputs = np.ascontiguousarray(outputs, dtype=np.float32)
    features = np.ascontiguousarray(features, dtype=np.float32)
    Y = np.ascontiguousarray(Y, dtype=np.float32)
    predicted_score = np.ascontiguousarray(predicted_score, dtype=np.float32)
    F2 = features.reshape(B, V * D)
    iota = np.tile(np.arange(CP, dtype=np.float32), (128, 1))
    ci8 = (np.arange(8, dtype=np.float32)[None, :] * 128
           + np.arange(128, dtype=np.float32)[:, None])
    ident = np.eye(128, dtype=np.float32)
    in_maps = []
    for m in range(NCORE):
        sl = slice(m * BL, (m + 1) * BL)
        in_maps.append({
            "f_full": F2,
            "f_loc": np.ascontiguousarray(F2[sl]),
            "o_loc": np.ascontiguousarray(outputs[sl]),
            "y_loc": np.ascontiguousarray(Y[sl]),
            "ps_loc": np.ascontiguousarray(predicted_score[sl]),
            "iota_in": iota,
            "ci8_in": ci8,
            "ident_in": ident,
        })
    return in_maps


def kernel(outputs, features, Y, predicted_score):
    nc = get_nc()
    in_maps = make_in_maps(outputs, features, Y, predicted_score)
    res = bass_utils.run_bass_kernel_spmd(
        nc, in_maps, core_ids=list(range(NCORE)), trace=False)
    nt = np.concatenate([res.results[m]["nt_out"] for m in range(NCORE)], axis=0)
    loss = np.float32(res.results[0]["loss_out"][0, 0])
    return loss, nt


if __name__ == "__main__":
    nc = get_nc()
    print("compiled OK")


# revision 16
# speedup vs baseline: 1.2002x; 1.2002x over previous
# Trainium2 Bass kernel for nn_ConLoss_798863917356 (partial-label supervised
# contrastive loss). 8-core SPMD, batch (B=2048) sharded 256 samples/core.
#
# Math (equivalent to the jax reference):
#   contrast = view-major stack of features        [N=4096, D=256]
#   adc = contrast @ contrast.T / T; mx[i] = adc[i,i] (= row max, unit vectors)
#   t[q] = argmax_c Y[q,c]*softmax(o)[q,c] = argmax over candidates of o[q,c]
#   cnt = bincount(t); Mw[p,q] = Y[p,t_q]*ps[p,t_q]/cnt[t_q]
#        = row p of (A*diag(1/max(cnt,1))) @ onehot(t)^T,  A = Y*ps
#   mean_log_prob_pos[i] = sum_q Mw[P,q]*(adc'[i,q]+adc'[i,b+q])
#                          - (mx'[i]+lse[i])*2*S[P] + Md[P]*lse[i]
#     where adc' = adc/T, mx' = diag/T, lse = ln(sum_{j!=i} exp(adc'[i,j]-mx'))
#           S[P] = sum_q Mw[P,q],  Md[P] = Mw[P,P] = A[P,t_P]/cnt[t_P]
#   loss = -mean(mean_log_prob_pos);  new_target = Y*exp(o-mxc)/rowsum(...)
#
# Collectives: one AllGather of [t_local(256) | counts_local(1024)] fp32,
# one AllGather of the per-core partial loss.
import sys

for _p in ("/opt/trn_rl_repo", "/root/.axon_site/_ro/trn_rl_repo"):
    if _p not in sys.path:
        sys.path.insert(0, _p)

import numpy as np

import concourse.bacc as bacc
import concourse.bass as bass
import concourse.mybir as mybir
import concourse.tile as tile
from concourse import bass_utils

F32 = mybir.dt.float32
F32R = mybir.dt.float32r
U32 = mybir.dt.uint32
AL = mybir.AluOpType
AF = mybir.ActivationFunctionType
AX = mybir.AxisListType

B, V, D, C = 2048, 2, 256, 1000
T = 0.07
NCORE = 8
BL = B // NCORE          # 256 local samples
RL = V * BL              # 512 local rows
N = V * B                # 4096
CP = 1024                # padded class count
BIG = 30000.0            # candidate mask offset (Y*BIG-BIG is exact for Y in {0,1})
CAT = BL + CP            # 1280 AllGather payload per core


def r(ap):
    """View an fp32 AP as float32r for fast PE matmuls."""
    return ap.bitcast(F32R)


def build_nc(collectives=True):
    # collectives=False replaces AllGathers with local DMA stand-ins so the
    # single-core TimelineSim (cost model) can run; used only for perf work.
    nc = bacc.Bacc(None, target_bir_lowering=False, debug=False,
                   num_devices=NCORE if collectives else 1)

    f_full = nc.dram_tensor("f_full", [B, V * D], F32, kind="ExternalInput")
    f_loc = nc.dram_tensor("f_loc", [BL, V * D], F32, kind="ExternalInput")
    o_loc = nc.dram_tensor("o_loc", [BL, C], F32, kind="ExternalInput")
    y_loc = nc.dram_tensor("y_loc", [BL, C], F32, kind="ExternalInput")
    ps_loc = nc.dram_tensor("ps_loc", [BL, C], F32, kind="ExternalInput")
    iota_in = nc.dram_tensor("iota_in", [128, CP], F32, kind="ExternalInput")
    ci8_in = nc.dram_tensor("ci8_in", [128, 8], F32, kind="ExternalInput")
    ident_in = nc.dram_tensor("ident_in", [128, 128], F32, kind="ExternalInput")
    nt_out = nc.dram_tensor("nt_out", [BL, C], F32, kind="ExternalOutput")
    loss_out = nc.dram_tensor("loss_out", [1, 1], F32, kind="ExternalOutput")

    with tile.TileContext(nc) as tc:
        with (
            tc.tile_pool(name="pc", bufs=1) as pc,
            tc.tile_pool(name="pk", bufs=1) as pk,
            tc.tile_pool(name="pdram", bufs=1, space="DRAM") as pdram,
            tc.tile_pool(name="ps_tr", bufs=2, space="PSUM") as ps_tr,
            tc.tile_pool(name="ps_mm", bufs=3, space="PSUM") as ps_mm,
            tc.tile_pool(name="ps_sm", bufs=3, space="PSUM") as ps_sm,
        ):
            # ---- long-lived constants / small vectors --------------------
            iota_sb = pc.tile([128, CP], F32, name="iota_sb")
            nc.sync.dma_start(iota_sb[:], iota_in[:])
            ci8 = pc.tile([128, 8], F32, name="ci8")
            nc.sync.dma_start(ci8[:], ci8_in[:])
            ident = pc.tile([128, 128], F32, name="ident")
            nc.sync.dma_start(ident[:], ident_in[:])
            ones_col = pc.tile([128, 1], F32, name="ones_col")
            nc.vector.memset(ones_col[:], 1.0)

            fl_sb = [pc.tile([128, V * D], F32, name=f"fl{pt}") for pt in range(2)]
            for pt in range(2):
                nc.sync.dma_start(fl_sb[pt][:], f_loc[pt * 128:(pt + 1) * 128, :])

            A = [pk.tile([128, CP], F32, name=f"A{pt}") for pt in range(2)]
            Ol = [pk.tile([128, CP], F32, name=f"Ol{pt}") for pt in range(2)]
            tl_f = [pc.tile([128, 1], F32, name=f"tl{pt}") for pt in range(2)]
            diag4 = pc.tile([128, 4], F32, name="diag4")
            negdT = pc.tile([128, 4], F32, name="negdT")
            Zall = pc.tile([128, 4, 8], F32, name="Zall")
            Dall = pc.tile([128, 4, 8], F32, name="Dall")
            Dacc4 = pc.tile([128, 4], F32, name="Dacc4")
            cl_row = pc.tile([1, CP], F32, name="cl_row")

            CT = [pk.tile([128, 2, B], F32, name=f"CT{k2}") for k2 in range(2)]
            CLT = [pk.tile([128, RL], F32, name=f"CLT{k2}") for k2 in range(2)]
            AT = [pk.tile([128, BL], F32, name=f"AT{k}") for k in range(8)]
            Mw = [pk.tile([128, B], F32, name=f"Mw{pt}") for pt in range(2)]
            t_bc = pk.tile([128, B], F32, name="t_bc")

            # DRAM scratch for collectives
            cat_in = pdram.tile([CAT], F32, name="cat_in")
            cat_out = pdram.tile([NCORE, CAT], F32, name="cat_out",
                                 addr_space="Shared" if collectives else "Local")
            cnt_lin = pdram.tile([CP], F32, name="cnt_lin")
            pl_in = pdram.tile([1], F32, name="pl_in")
            pl_out = pdram.tile([NCORE], F32, name="pl_out",
                                addr_space="Shared" if collectives else "Local")

            # ================= stage A: argmax / new_target / A ============
            with tc.tile_pool(name="pe1", bufs=1) as pe1:
                usum = pc.tile([128, 2], F32, name="usum")
                for pt in range(2):
                    sl = slice(pt * 128, (pt + 1) * 128)
                    o_sb = pe1.tile([128, C], F32, name=f"o_sb{pt}")
                    nc.sync.dma_start(o_sb[:], o_loc[sl, :])
                    y_sb = pe1.tile([128, C], F32, name=f"y_sb{pt}")
                    nc.sync.dma_start(y_sb[:], y_loc[sl, :])
                    p_sb = pe1.tile([128, C], F32, name=f"p_sb{pt}")
                    nc.sync.dma_start(p_sb[:], ps_loc[sl, :])

                    # masked = o + (Y-1)*BIG   (exact for candidates)
                    tmp = pe1.tile([128, C], F32, name=f"tmp{pt}")
                    nc.vector.tensor_scalar(tmp[:], y_sb[:], BIG, -BIG, AL.mult, AL.add)
                    masked = pe1.tile([128, C], F32, name=f"msk{pt}")
                    nc.vector.tensor_tensor(masked[:], tmp[:], o_sb[:], AL.add)

                    mx8 = pe1.tile([128, 8], F32, name=f"mx8{pt}")
                    idx8 = pe1.tile([128, 8], U32, name=f"idx8{pt}")
                    nc.vector.max_with_indices(mx8[:], idx8[:], masked[:])
                    nc.vector.tensor_copy(tl_f[pt][:], idx8[:, 0:1])

                    negmx = pe1.tile([128, 1], F32, name=f"negmx{pt}")
                    nc.vector.tensor_scalar_mul(negmx[:], mx8[:, 0:1], -1.0)

                    # u = exp(masked - mx)  (0 exactly for non-candidates)
                    u = pe1.tile([128, C], F32, name=f"u{pt}")
                    nc.scalar.activation(u[:], masked[:], AF.Exp,
                                         bias=negmx[:, 0:1], scale=1.0,
                                         accum_out=usum[:, pt:pt + 1])
                    rus = pe1.tile([128, 1], F32, name=f"rus{pt}")
                    nc.vector.reciprocal(rus[:], usum[:, pt:pt + 1])
                    nt_t = pe1.tile([128, C], F32, name=f"nt{pt}")
                    nc.vector.tensor_scalar(nt_t[:], u[:], rus[:, 0:1], None, AL.mult)
                    nc.sync.dma_start(nt_out[sl, :], nt_t[:])

                    # A = Y*ps (padded with zeros), Ol = onehot(t_local)
                    nc.vector.memset(A[pt][:, C:CP], 0.0)
                    nc.vector.tensor_tensor(A[pt][:, 0:C], y_sb[:], p_sb[:], AL.mult)
                    nc.vector.tensor_scalar(Ol[pt][:], iota_sb[:], tl_f[pt][:, 0:1],
                                            None, AL.is_equal)

                # local counts: cnt_loc[c] = sum_q Ol[q, c]  (PE partition-sum)
                for h in range(2):
                    cnt_ps = ps_sm.tile([1, 512], F32, name="cnt_ps", tag="sm")
                    for pt in range(2):
                        nc.tensor.matmul(cnt_ps[:],
                                         ones_col[:],
                                         Ol[pt][:, h * 512:(h + 1) * 512],
                                         start=(pt == 0), stop=(pt == 1))
                    nc.scalar.copy(cl_row[:, h * 512:(h + 1) * 512], cnt_ps[:])

                # ship [t_local | counts_local] through one AllGather
                for pt in range(2):
                    nc.sync.dma_start(cat_in[pt * 128:(pt + 1) * 128], tl_f[pt][:])
                nc.sync.dma_start(cat_in[BL:CAT], cl_row[:])
                if collectives:
                    nc.gpsimd.collective_compute(
                        "AllGather", AL.bypass,
                        replica_groups=[list(range(NCORE))],
                        ins=[cat_in[:]], outs=[cat_out[:]],
                    )
                else:
                    for rr in range(NCORE):
                        nc.sync.dma_start(cat_out[rr, :], cat_in[:])

                # ---- transposes: CLT (local) and CT (full), d on partitions
                for k2 in range(2):
                    psC = ps_tr.tile([128, 512], F32, name="psC", tag="tr")
                    for a in range(2):
                        for pt in range(2):
                            nc.tensor.transpose(
                                psC[:, a * 256 + pt * 128:a * 256 + pt * 128 + 128],
                                fl_sb[pt][:, (a * 2 + k2) * 128:(a * 2 + k2) * 128 + 128],
                                ident[:])
                    nc.scalar.copy(r(CLT[k2][:]), psC[:])

                for s in range(16):
                    fs = pe1.tile([128, V * D], F32, name="fs", tag="fs", bufs=3)
                    nc.sync.dma_start(fs[:], f_full[s * 128:(s + 1) * 128, :])
                    for k2 in range(2):
                        psT = ps_tr.tile([128, 512], F32, name="psT", tag="tr")
                        for c in range(2):
                            nc.tensor.transpose(
                                psT[:, c * 128:(c + 1) * 128],
                                fs[:, (c * 2 + k2) * 128:(c * 2 + k2) * 128 + 128],
                                ident[:])
                        # psT[:, c*128:...] -> CT[k2][:, c, s*128:...]
                        nc.scalar.copy(
                            r(CT[k2][:, :, s * 128:(s + 1) * 128]),
                            psT[:, 0:256].rearrange("p (c q) -> p c q", c=2))

                # ---- diag via local-block matmul (same rounding as sweep)
                for rt in range(4):
                    psD = ps_mm.tile([128, 512], F32, name="psD", tag="mm")
                    for k2 in range(2):
                        nc.tensor.matmul(psD[:],
                                         r(CLT[k2][:, rt * 128:(rt + 1) * 128]),
                                         r(CLT[k2][:]),
                                         start=(k2 == 0), stop=(k2 == 1))
                    scrI = pe1.tile([128, 128], F32, name="scrI", tag="scrI", bufs=2)
                    nc.vector.scalar_tensor_tensor(
                        scrI[:], psD[:, rt * 128:(rt + 1) * 128], 1.0, ident[:],
                        AL.mult, AL.mult, accum_out=diag4[:, rt:rt + 1])
                nc.vector.tensor_scalar_mul(negdT[:], diag4[:], -1.0 / T)

            # ================= stage B: post-AllGather =====================
            with tc.tile_pool(name="pe2", bufs=1) as pe2:
                t_row = pe2.tile([1, B], F32, name="t_row")
                nc.sync.dma_start(
                    t_row[:].rearrange("p (r x) -> p r x", r=NCORE),
                    cat_out[:, 0:BL])
                nc.gpsimd.partition_broadcast(t_bc[:], t_row[0:1, :])

                cntp = pe2.tile([128, 8, 8], F32, name="cntp")
                for rr in range(NCORE):
                    nc.sync.dma_start(
                        cntp[:, :, rr:rr + 1],
                        cat_out[rr:rr + 1, BL:CAT].rearrange("r (k p) -> p k r", p=128))
                cnt8 = pe2.tile([128, 8], F32, name="cnt8")
                with nc.allow_low_precision(reason="counts are small ints, exact in f32r"):
                    nc.vector.reduce_sum(r(cnt8[:]), cntp[:], axis=AX.X)
                c1 = pe2.tile([128, 8], F32, name="c1")
                nc.vector.tensor_scalar(c1[:], cnt8[:], 1.0, None, AL.max)
                invc8 = pe2.tile([128, 8], F32, name="invc8")
                nc.vector.reciprocal(invc8[:], c1[:])

                # cnt broadcast row for Md gather
                nc.sync.dma_start(cnt_lin[:].rearrange("(k p) -> p k", p=128), cnt8[:])
                cnt_row = pe2.tile([1, CP], F32, name="cnt_row")
                nc.sync.dma_start(cnt_row[:], cnt_lin[:])
                cnt_bc = pe2.tile([128, CP], F32, name="cnt_bc")
                nc.gpsimd.partition_broadcast(cnt_bc[:], cnt_row[0:1, :])

                # Md[p] = A[p, t_l]/cnt[t_l]
                Md = pe2.tile([128, 2], F32, name="Md")
                Ad = pe2.tile([128, 2], F32, name="Ad")
                ctl = pe2.tile([128, 2], F32, name="ctl")
                for pt in range(2):
                    scrC = pe2.tile([128, CP], F32, name="scrC", tag="scrC", bufs=2)
                    nc.vector.scalar_tensor_tensor(
                        scrC[:], A[pt][:], 1.0, Ol[pt][:],
                        AL.mult, AL.mult, accum_out=Ad[:, pt:pt + 1])
                    scrC2 = pe2.tile([128, CP], F32, name="scrC2", tag="scrC", bufs=2)
                    nc.vector.scalar_tensor_tensor(
                        scrC2[:], cnt_bc[:], 1.0, Ol[pt][:],
                        AL.mult, AL.mult, accum_out=ctl[:, pt:pt + 1])
                rctl = pe2.tile([128, 2], F32, name="rctl")
                nc.vector.reciprocal(rctl[:], ctl[:])
                nc.vector.tensor_tensor(Md[:], Ad[:], rctl[:], AL.mult)

                # O^T tiles: OT[k][c, q] = (t_q == k*128+c)
                OT = [pe2.tile([128, B], F32, name=f"OT{k}") for k in range(8)]
                for k in range(8):
                    nc.vector.tensor_scalar(r(OT[k][:]), t_bc[:], ci8[:, k:k + 1],
                                            None, AL.is_equal)

                # A'T[k] = transpose(A)[k-tile] * invcnt (scaled on evac)
                for pt in range(2):
                    for k in range(8):
                        psA = ps_tr.tile([128, 512], F32, name="psA", tag="tr")
                        nc.tensor.transpose(
                            psA[:, 0:128],
                            A[pt][:, k * 128:(k + 1) * 128], ident[:])
                        nc.scalar.activation(r(AT[k][:, pt * 128:(pt + 1) * 128]),
                                             psA[:, 0:128], AF.Copy,
                                             bias=0.0, scale=invc8[:, k:k + 1])

                # S[p] = sum_c A'T[c,p]*cnt[c]
                Ssb = pe2.tile([128, 2], F32, name="Ssb")
                for pt in range(2):
                    psS = ps_sm.tile([128, 1], F32, name="psS", tag="sm")
                    for k in range(8):
                        nc.tensor.matmul(psS[:],
                                         AT[k][:, pt * 128:(pt + 1) * 128],
                                         cnt8[:, k:k + 1],
                                         start=(k == 0), stop=(k == 7))
                    nc.scalar.copy(Ssb[:, pt:pt + 1], psS[:])

                # Mw[pt] = A'T.T @ OT   [128, 2048]
                for pt in range(2):
                    for n4 in range(4):
                        psM = ps_mm.tile([128, 512], F32, name="psM", tag="mm")
                        for k in range(8):
                            nc.tensor.matmul(psM[:],
                                             r(AT[k][:, pt * 128:(pt + 1) * 128]),
                                             r(OT[k][:, n4 * 512:(n4 + 1) * 512]),
                                             start=(k == 0), stop=(k == 7))
                        nc.scalar.copy(Mw[pt][:, n4 * 512:(n4 + 1) * 512], psM[:])

                # ---- adc sweep: Z (ACT exp accum) + D (DVE fused mul-reduce)
                for rt in range(4):
                    pt = rt % 2
                    for ct in range(8):
                        c = ct // 4
                        q4 = ct % 4
                        psmm = ps_mm.tile([128, 512], F32, name="psmm", tag="mm")
                        for k2 in range(2):
                            nc.tensor.matmul(
                                psmm[:],
                                r(CLT[k2][:, rt * 128:(rt + 1) * 128]),
                                r(CT[k2][:, c, q4 * 512:(q4 + 1) * 512]),
                                start=(k2 == 0), stop=(k2 == 1))
                        expo = pe2.tile([128, 512], F32, name="expo", tag="expo", bufs=4)
                        nc.scalar.activation(expo[:], psmm[:], AF.Exp,
                                             bias=negdT[:, rt:rt + 1], scale=1.0 / T,
                                             accum_out=Zall[:, rt, ct:ct + 1])
                        scrT = pe2.tile([128, 512], F32, name="scrT", tag="scrT", bufs=4)
                        nc.vector.scalar_tensor_tensor(
                            scrT[:], psmm[:], 1.0 / T,
                            Mw[pt][:, q4 * 512:(q4 + 1) * 512],
                            AL.mult, AL.mult,
                            accum_out=Dall[:, rt, ct:ct + 1])

                # ---- final per-row combine ------------------------------
                Zrow = pe2.tile([128, 4], F32, name="Zrow")
                nc.vector.reduce_sum(Zrow[:], Zall[:], axis=AX.X)
                nc.vector.reduce_sum(Dacc4[:], Dall[:], axis=AX.X)
                Zm = pe2.tile([128, 4], F32, name="Zm")
                nc.vector.tensor_scalar(Zm[:], Zrow[:], -1.0, None, AL.add)
                lse4 = pe2.tile([128, 4], F32, name="lse4")
                nc.scalar.activation(lse4[:], Zm[:], AF.Ln)

                S4 = pe2.tile([128, 4], F32, name="S4")
                Md4 = pe2.tile([128, 4], F32, name="Md4")
                S2x = pe2.tile([128, 2], F32, name="S2x")
                nc.vector.tensor_scalar_mul(S2x[:], Ssb[:], 2.0)
                for a in range(2):
                    nc.vector.tensor_copy(S4[:, a * 2:a * 2 + 2], S2x[:])
                    nc.vector.tensor_copy(Md4[:, a * 2:a * 2 + 2], Md[:])

                tt = pe2.tile([128, 4], F32, name="tt")
                nc.vector.tensor_tensor(tt[:], lse4[:], negdT[:], AL.subtract)
                m2 = pe2.tile([128, 4], F32, name="m2")
                nc.vector.tensor_tensor(m2[:], tt[:], S4[:], AL.mult)
                m3 = pe2.tile([128, 4], F32, name="m3")
                nc.vector.tensor_tensor(m3[:], Dacc4[:], m2[:], AL.subtract)
                m4 = pe2.tile([128, 4], F32, name="m4")
                nc.vector.tensor_tensor(m4[:], Md4[:], lse4[:], AL.mult)
                mlpp = pe2.tile([128, 4], F32, name="mlpp")
                nc.vector.tensor_tensor(mlpp[:], m3[:], m4[:], AL.add)

                psL = ps_sm.tile([1, 4], F32, name="psL", tag="sm")
                nc.tensor.matmul(psL[:], ones_col[:], mlpp[:], start=True, stop=True)
                Lrow = pe2.tile([1, 4], F32, name="Lrow")
                nc.scalar.copy(Lrow[:], psL[:])
                pl = pe2.tile([1, 1], F32, name="pl")
                nc.vector.reduce_sum(pl[:], Lrow[:], axis=AX.X)
                nc.sync.dma_start(pl_in[:], pl[:])
                if collectives:
                    nc.gpsimd.collective_compute(
                        "AllGather", AL.bypass,
                        replica_groups=[list(range(NCORE))],
                        ins=[pl_in[:]], outs=[pl_out[:]],
                    )
                else:
                    for rr in range(NCORE):
                        nc.sync.dma_start(pl_out[rr:rr + 1], pl_in[:])
                plr = pe2.tile([1, NCORE], F32, name="plr")
                nc.sync.dma_start(plr[:], pl_out[:])
                ptot = pe2.tile([1, 1], F32, name="ptot")
                nc.vector.reduce_sum(ptot[:], plr[:], axis=AX.X)
                lossv = pe2.tile([1, 1], F32, name="lossv")
                nc.vector.tensor_scalar_mul(lossv[:], ptot[:], -1.0 / N)
                nc.sync.dma_start(loss_out[:], lossv[:])

    nc.compile()
    return nc


_CACHE = {}


def get_nc():
    if "nc" not in _CACHE:
        _CACHE["nc"] = build_nc()
    return _CACHE["nc"]


def make_in_maps(outputs, features, Y, predicted_score):
    outputs = np.ascontiguousarray(outputs, dtype=np.float32)
    features = np.ascontiguousarray(features, dtype=np.float32)
    Y = np.ascontiguousarray(Y, dtype=np.float32)
    predicted_score = np.ascontiguousarray(predicted_score, dtype=np.float32)
    F2 = features.reshape(B, V * D)
    iota = np.tile(np.arange(CP, dtype=np.float32), (128, 1))
    ci8 = (np.arange(8, dtype=np.float32)[None, :] * 128
           + np.arange(128, dtype=np.float32)[:, None])
    ident = np.eye(128, dtype=np.float32)
    in_maps = []
    for m in range(NCORE):
        sl = slice(m * BL, (m + 1) * BL)
        in_maps.append({
            "f_full": F2,
            "f_loc": np.ascontiguousarray(F2[sl]),
            "o_loc": np.ascontiguousarray(outputs[sl]),
            "y_loc": np.ascontiguousarray(Y[sl]),
            "ps_loc": np.ascontiguousarray(predicted_score[sl]),
            "iota_in": iota,
            "ci8_in": ci8,
            "ident_in": ident,
        })
    return in_maps


def kernel(outputs, features, Y, predicted_score):
    nc = get_nc()
    in_maps = make_in_maps(outputs, features, Y, predicted_score)
    res = bass_utils.run_bass_kernel_spmd(
        nc, in_maps, core_ids=list(range(NCORE)), trace=False)
    nt = np.concatenate([res.results[m]["nt_out"] for m in range(NCORE)], axis=0)
    loss = np.float32(res.results[0]["loss_out"][0, 0])
    return loss, nt


if __name__ == "__main__":
    nc = get_nc()
    print("compiled OK")


# revision 17
# speedup vs baseline: 1.3499x; 1.1247x over previous
# Trainium2 Bass kernel for nn_ConLoss_798863917356 (partial-label supervised
# contrastive loss). 8-core SPMD, batch (B=2048) sharded 256 samples/core.
#
# Math (equivalent to the jax reference):
#   contrast = view-major stack of features        [N=4096, D=256]
#   adc = contrast @ contrast.T / T; mx[i] = adc[i,i] (= row max, unit vectors)
#   t[q] = argmax_c Y[q,c]*softmax(o)[q,c] = argmax over candidates of o[q,c]
#   cnt = bincount(t); Mw[p,q] = Y[p,t_q]*ps[p,t_q]/cnt[t_q]
#        = row p of (A*diag(1/max(cnt,1))) @ onehot(t)^T,  A = Y*ps
#   mean_log_prob_pos[i] = sum_q Mw[P,q]*(adc'[i,q]+adc'[i,b+q])
#                          - (mx'[i]+lse[i])*2*S[P] + Md[P]*lse[i]
#     where adc' = adc/T, mx' = diag/T, lse = ln(sum_{j!=i} exp(adc'[i,j]-mx'))
#           S[P] = sum_q Mw[P,q],  Md[P] = Mw[P,P] = A[P,t_P]/cnt[t_P]
#   loss = -mean(mean_log_prob_pos);  new_target = Y*exp(o-mxc)/rowsum(...)
#
# Collectives: one AllGather of [t_local(256) | counts_local(1024)] fp32,
# one AllGather of the per-core partial loss.
import sys

for _p in ("/opt/trn_rl_repo", "/root/.axon_site/_ro/trn_rl_repo"):
    if _p not in sys.path:
        sys.path.insert(0, _p)

import numpy as np

import concourse.bacc as bacc
import concourse.bass as bass
import concourse.mybir as mybir
import concourse.tile as tile
from concourse import bass_utils

F32 = mybir.dt.float32
F32R = mybir.dt.float32r
U32 = mybir.dt.uint32
AL = mybir.AluOpType
AF = mybir.ActivationFunctionType
AX = mybir.AxisListType

B, V, D, C = 2048, 2, 256, 1000
T = 0.07
NCORE = 8
BL = B // NCORE          # 256 local samples
RL = V * BL              # 512 local rows
N = V * B                # 4096
CP = 1024                # padded class count
BIG = 30000.0            # candidate mask offset (Y*BIG-BIG is exact for Y in {0,1})
CAT = BL + CP            # 1280 AllGather payload per core


def r(ap):
    """View an fp32 AP as float32r for fast PE matmuls."""
    return ap.bitcast(F32R)


def build_nc(collectives=True):
    # collectives=False replaces AllGathers with local DMA stand-ins so the
    # single-core TimelineSim (cost model) can run; used only for perf work.
    nc = bacc.Bacc(None, target_bir_lowering=False, debug=False,
                   num_devices=NCORE if collectives else 1)

    f_full = nc.dram_tensor("f_full", [B, V * D], F32, kind="ExternalInput")
    f_loc = nc.dram_tensor("f_loc", [BL, V * D], F32, kind="ExternalInput")
    o_loc = nc.dram_tensor("o_loc", [BL, C], F32, kind="ExternalInput")
    y_loc = nc.dram_tensor("y_loc", [BL, C], F32, kind="ExternalInput")
    ps_loc = nc.dram_tensor("ps_loc", [BL, C], F32, kind="ExternalInput")
    iota_in = nc.dram_tensor("iota_in", [128, CP], F32, kind="ExternalInput")
    ci8_in = nc.dram_tensor("ci8_in", [128, 8], F32, kind="ExternalInput")
    ident_in = nc.dram_tensor("ident_in", [128, 128], F32, kind="ExternalInput")
    nt_out = nc.dram_tensor("nt_out", [BL, C], F32, kind="ExternalOutput")
    loss_out = nc.dram_tensor("loss_out", [1, 1], F32, kind="ExternalOutput")

    with tile.TileContext(nc) as tc:
        with (
            tc.tile_pool(name="pc", bufs=1) as pc,
            tc.tile_pool(name="pk", bufs=1) as pk,
            tc.tile_pool(name="pdram", bufs=1, space="DRAM") as pdram,
            tc.tile_pool(name="ps_tr", bufs=2, space="PSUM") as ps_tr,
            tc.tile_pool(name="ps_mm", bufs=3, space="PSUM") as ps_mm,
            tc.tile_pool(name="ps_sm", bufs=3, space="PSUM") as ps_sm,
        ):
            # ---- long-lived constants / small vectors --------------------
            iota_sb = pc.tile([128, CP], F32, name="iota_sb")
            nc.sync.dma_start(iota_sb[:], iota_in[:])
            ci8 = pc.tile([128, 8], F32, name="ci8")
            nc.sync.dma_start(ci8[:], ci8_in[:])
            ident = pc.tile([128, 128], F32, name="ident")
            nc.sync.dma_start(ident[:], ident_in[:])
            ones_col = pc.tile([128, 1], F32, name="ones_col")
            nc.vector.memset(ones_col[:], 1.0)

            fl_sb = [pc.tile([128, V * D], F32, name=f"fl{pt}") for pt in range(2)]
            for pt in range(2):
                nc.sync.dma_start(fl_sb[pt][:], f_loc[pt * 128:(pt + 1) * 128, :])

            A = [pk.tile([128, CP], F32, name=f"A{pt}") for pt in range(2)]
            Ol = [pk.tile([128, CP], F32, name=f"Ol{pt}") for pt in range(2)]
            tl_f = [pc.tile([128, 1], F32, name=f"tl{pt}") for pt in range(2)]
            diag4 = pc.tile([128, 4], F32, name="diag4")
            negdT = pc.tile([128, 4], F32, name="negdT")
            Zall = pc.tile([128, 4, 8], F32, name="Zall")
            Dall = pc.tile([128, 4, 8], F32, name="Dall")
            Dacc4 = pc.tile([128, 4], F32, name="Dacc4")
            cl_row = pc.tile([1, CP], F32, name="cl_row")

            CT = [pk.tile([128, 2, B], F32, name=f"CT{k2}") for k2 in range(2)]
            CLT = [pk.tile([128, RL], F32, name=f"CLT{k2}") for k2 in range(2)]
            AT = [pk.tile([128, BL], F32, name=f"AT{k}") for k in range(8)]
            Mw = [pk.tile([128, B], F32, name=f"Mw{pt}") for pt in range(2)]
            t_bc = pk.tile([128, B], F32, name="t_bc")

            # DRAM scratch for collectives
            cat_in = pdram.tile([CAT], F32, name="cat_in")
            cat_out = pdram.tile([NCORE, CAT], F32, name="cat_out",
                                 addr_space="Shared" if collectives else "Local")
            cnt_lin = pdram.tile([CP], F32, name="cnt_lin")
            pl_in = pdram.tile([1], F32, name="pl_in")
            pl_out = pdram.tile([NCORE], F32, name="pl_out",
                                addr_space="Shared" if collectives else "Local")

            # ================= stage A: argmax / new_target / A ============
            with tc.tile_pool(name="pe1", bufs=1) as pe1:
                usum = pc.tile([128, 2], F32, name="usum")
                for pt in range(2):
                    sl = slice(pt * 128, (pt + 1) * 128)
                    o_sb = pe1.tile([128, C], F32, name=f"o_sb{pt}")
                    nc.sync.dma_start(o_sb[:], o_loc[sl, :])
                    y_sb = pe1.tile([128, C], F32, name=f"y_sb{pt}")
                    nc.sync.dma_start(y_sb[:], y_loc[sl, :])
                    p_sb = pe1.tile([128, C], F32, name=f"p_sb{pt}")
                    nc.sync.dma_start(p_sb[:], ps_loc[sl, :])

                    # masked = o + (Y-1)*BIG   (exact for candidates)
                    tmp = pe1.tile([128, C], F32, name=f"tmp{pt}")
                    nc.vector.tensor_scalar(tmp[:], y_sb[:], BIG, -BIG, AL.mult, AL.add)
                    masked = pe1.tile([128, C], F32, name=f"msk{pt}")
                    nc.vector.tensor_tensor(masked[:], tmp[:], o_sb[:], AL.add)

                    mx8 = pe1.tile([128, 8], F32, name=f"mx8{pt}")
                    idx8 = pe1.tile([128, 8], U32, name=f"idx8{pt}")
                    nc.vector.max_with_indices(mx8[:], idx8[:], masked[:])
                    nc.vector.tensor_copy(tl_f[pt][:], idx8[:, 0:1])

                    negmx = pe1.tile([128, 1], F32, name=f"negmx{pt}")
                    nc.vector.tensor_scalar_mul(negmx[:], mx8[:, 0:1], -1.0)

                    # u = exp(masked - mx)  (0 exactly for non-candidates)
                    u = pe1.tile([128, C], F32, name=f"u{pt}")
                    nc.scalar.activation(u[:], masked[:], AF.Exp,
                                         bias=negmx[:, 0:1], scale=1.0,
                                         accum_out=usum[:, pt:pt + 1])
                    rus = pe1.tile([128, 1], F32, name=f"rus{pt}")
                    nc.vector.reciprocal(rus[:], usum[:, pt:pt + 1])
                    nt_t = pe1.tile([128, C], F32, name=f"nt{pt}")
                    nc.vector.tensor_scalar(nt_t[:], u[:], rus[:, 0:1], None, AL.mult)
                    nc.sync.dma_start(nt_out[sl, :], nt_t[:])

                    # A = Y*ps (padded with zeros), Ol = onehot(t_local)
                    nc.vector.memset(A[pt][:, C:CP], 0.0)
                    nc.vector.tensor_tensor(A[pt][:, 0:C], y_sb[:], p_sb[:], AL.mult)
                    nc.vector.tensor_scalar(Ol[pt][:], iota_sb[:], tl_f[pt][:, 0:1],
                                            None, AL.is_equal)

                # local counts: cnt_loc[c] = sum_q Ol[q, c]  (PE partition-sum)
                for h in range(2):
                    cnt_ps = ps_sm.tile([1, 512], F32, name="cnt_ps", tag="sm")
                    for pt in range(2):
                        nc.tensor.matmul(cnt_ps[:],
                                         ones_col[:],
                                         Ol[pt][:, h * 512:(h + 1) * 512],
                                         start=(pt == 0), stop=(pt == 1))
                    nc.scalar.copy(cl_row[:, h * 512:(h + 1) * 512], cnt_ps[:])

                # ship [t_local | counts_local] through one AllGather
                for pt in range(2):
                    nc.sync.dma_start(cat_in[pt * 128:(pt + 1) * 128], tl_f[pt][:])
                nc.sync.dma_start(cat_in[BL:CAT], cl_row[:])
                if collectives:
                    nc.gpsimd.collective_compute(
                        "AllGather", AL.bypass,
                        replica_groups=[list(range(NCORE))],
                        ins=[cat_in[:]], outs=[cat_out[:]],
                    )
                else:
                    for rr in range(NCORE):
                        nc.sync.dma_start(cat_out[rr, :], cat_in[:])

                # ---- transposes: CLT (local) and CT (full), d on partitions
                for k2 in range(2):
                    psC = ps_tr.tile([128, 512], F32, name="psC", tag="tr")
                    for a in range(2):
                        for pt in range(2):
                            nc.tensor.transpose(
                                psC[:, a * 256 + pt * 128:a * 256 + pt * 128 + 128],
                                fl_sb[pt][:, (a * 2 + k2) * 128:(a * 2 + k2) * 128 + 128],
                                ident[:])
                    nc.scalar.copy(r(CLT[k2][:]), psC[:])

                for s in range(16):
                    fs = pe1.tile([128, V * D], F32, name="fs", tag="fs", bufs=3)
                    nc.sync.dma_start(fs[:], f_full[s * 128:(s + 1) * 128, :])
                    for k2 in range(2):
                        psT = ps_tr.tile([128, 512], F32, name="psT", tag="tr")
                        for c in range(2):
                            nc.tensor.transpose(
                                psT[:, c * 128:(c + 1) * 128],
                                fs[:, (c * 2 + k2) * 128:(c * 2 + k2) * 128 + 128],
                                ident[:])
                        # psT[:, c*128:...] -> CT[k2][:, c, s*128:...]
                        nc.scalar.copy(
                            r(CT[k2][:, :, s * 128:(s + 1) * 128]),
                            psT[:, 0:256].rearrange("p (c q) -> p c q", c=2))

                # ---- diag via local-block matmul (same rounding as sweep)
                for rt in range(4):
                    psD = ps_mm.tile([128, 512], F32, name="psD", tag="mm")
                    for k2 in range(2):
                        nc.tensor.matmul(psD[:],
                                         r(CLT[k2][:, rt * 128:(rt + 1) * 128]),
                                         r(CLT[k2][:]),
                                         start=(k2 == 0), stop=(k2 == 1))
                    scrI = pe1.tile([128, 128], F32, name="scrI", tag="scrI", bufs=2)
                    nc.vector.scalar_tensor_tensor(
                        scrI[:], psD[:, rt * 128:(rt + 1) * 128], 1.0, ident[:],
                        AL.mult, AL.mult, accum_out=diag4[:, rt:rt + 1])
                nc.vector.tensor_scalar_mul(negdT[:], diag4[:], -1.0 / T)

            # ================= stage B: post-AllGather =====================
            with tc.tile_pool(name="pe2", bufs=1) as pe2:
                t_row = pe2.tile([1, B], F32, name="t_row")
                nc.sync.dma_start(
                    t_row[:].rearrange("p (r x) -> p r x", r=NCORE),
                    cat_out[:, 0:BL])
                nc.gpsimd.partition_broadcast(t_bc[:], t_row[0:1, :])

                cntp = pe2.tile([128, 8, 8], F32, name="cntp")
                for rr in range(NCORE):
                    nc.sync.dma_start(
                        cntp[:, :, rr:rr + 1],
                        cat_out[rr:rr + 1, BL:CAT].rearrange("r (k p) -> p k r", p=128))
                cnt8 = pe2.tile([128, 8], F32, name="cnt8")
                with nc.allow_low_precision(reason="counts are small ints, exact in f32r"):
                    nc.vector.reduce_sum(r(cnt8[:]), cntp[:], axis=AX.X)
                c1 = pe2.tile([128, 8], F32, name="c1")
                nc.vector.tensor_scalar(c1[:], cnt8[:], 1.0, None, AL.max)
                invc8 = pe2.tile([128, 8], F32, name="invc8")
                nc.vector.reciprocal(invc8[:], c1[:])

                # cnt broadcast row for Md gather
                nc.sync.dma_start(cnt_lin[:].rearrange("(k p) -> p k", p=128), cnt8[:])
                cnt_row = pe2.tile([1, CP], F32, name="cnt_row")
                nc.sync.dma_start(cnt_row[:], cnt_lin[:])
                cnt_bc = pe2.tile([128, CP], F32, name="cnt_bc")
                nc.gpsimd.partition_broadcast(cnt_bc[:], cnt_row[0:1, :])

                # Md[p] = A[p, t_l]/cnt[t_l]
                Md = pe2.tile([128, 2], F32, name="Md")
                Ad = pe2.tile([128, 2], F32, name="Ad")
                ctl = pe2.tile([128, 2], F32, name="ctl")
                for pt in range(2):
                    scrC = pe2.tile([128, CP], F32, name="scrC", tag="scrC", bufs=2)
                    nc.vector.scalar_tensor_tensor(
                        scrC[:], A[pt][:], 1.0, Ol[pt][:],
                        AL.mult, AL.mult, accum_out=Ad[:, pt:pt + 1])
                    scrC2 = pe2.tile([128, CP], F32, name="scrC2", tag="scrC", bufs=2)
                    nc.vector.scalar_tensor_tensor(
                        scrC2[:], cnt_bc[:], 1.0, Ol[pt][:],
                        AL.mult, AL.mult, accum_out=ctl[:, pt:pt + 1])
                rctl = pe2.tile([128, 2], F32, name="rctl")
                nc.vector.reciprocal(rctl[:], ctl[:])
                nc.vector.tensor_tensor(Md[:], Ad[:], rctl[:], AL.mult)

                # O^T tiles: OT[k][c, q] = (t_q == k*128+c)
                OT = [pe2.tile([128, B], F32, name=f"OT{k}") for k in range(8)]
                for k in range(8):
                    nc.vector.tensor_scalar(r(OT[k][:]), t_bc[:], ci8[:, k:k + 1],
                                            None, AL.is_equal)

                # A'T[k] = transpose(A)[k-tile] * invcnt (scaled on evac)
                for pt in range(2):
                    for k in range(8):
                        psA = ps_tr.tile([128, 512], F32, name="psA", tag="tr")
                        nc.tensor.transpose(
                            psA[:, 0:128],
                            A[pt][:, k * 128:(k + 1) * 128], ident[:])
                        nc.scalar.activation(r(AT[k][:, pt * 128:(pt + 1) * 128]),
                                             psA[:, 0:128], AF.Copy,
                                             bias=0.0, scale=invc8[:, k:k + 1])

                # S[p] = sum_c A'T[c,p]*cnt[c]
                Ssb = pe2.tile([128, 2], F32, name="Ssb")
                for pt in range(2):
                    psS = ps_sm.tile([128, 1], F32, name="psS", tag="sm")
                    for k in range(8):
                        nc.tensor.matmul(psS[:],
                                         AT[k][:, pt * 128:(pt + 1) * 128],
                                         cnt8[:, k:k + 1],
                                         start=(k == 0), stop=(k == 7))
                    nc.scalar.copy(Ssb[:, pt:pt + 1], psS[:])

                # Mw[pt] = A'T.T @ OT   [128, 2048]
                for pt in range(2):
                    for n4 in range(4):
                        psM = ps_mm.tile([128, 512], F32, name="psM", tag="mm")
                        for k in range(8):
                            nc.tensor.matmul(psM[:],
                                             r(AT[k][:, pt * 128:(pt + 1) * 128]),
                                             r(OT[k][:, n4 * 512:(n4 + 1) * 512]),
                                             start=(k == 0), stop=(k == 7))
                        nc.vector.tensor_copy(Mw[pt][:, n4 * 512:(n4 + 1) * 512], psM[:])

                # ---- adc sweep: Z (ACT exp accum) + D (DVE fused mul-reduce)
                for rt in range(4):
                    pt = rt % 2
                    for ct in range(8):
                        c = ct // 4
                        q4 = ct % 4
                        psmm = ps_mm.tile([128, 512], F32, name="psmm", tag="mm")
                        for k2 in range(2):
                            nc.tensor.matmul(
                                psmm[:],
                                r(CLT[k2][:, rt * 128:(rt + 1) * 128]),
                                r(CT[k2][:, c, q4 * 512:(q4 + 1) * 512]),
                                start=(k2 == 0), stop=(k2 == 1))
                        expo = pe2.tile([128, 512], F32, name="expo", tag="expo", bufs=4)
                        nc.scalar.activation(expo[:], psmm[:], AF.Exp,
                                             bias=negdT[:, rt:rt + 1], scale=1.0 / T,
                                             accum_out=Zall[:, rt, ct:ct + 1])
                        scrT = pe2.tile([128, 512], F32, name="scrT", tag="scrT", bufs=4)
                        nc.vector.scalar_tensor_tensor(
                            scrT[:], psmm[:], 1.0 / T,
                            Mw[pt][:, q4 * 512:(q4 + 1) * 512],
                            AL.mult, AL.mult,
                            accum_out=Dall[:, rt, ct:ct + 1])

                # ---- final per-row combine ------------------------------
                Zrow = pe2.tile([128, 4], F32, name="Zrow")
                nc.vector.reduce_sum(Zrow[:], Zall[:], axis=AX.X)
                nc.vector.reduce_sum(Dacc4[:], Dall[:], axis=AX.X)
                Zm = pe2.tile([128, 4], F32, name="Zm")
                nc.vector.tensor_scalar(Zm[:], Zrow[:], -1.0, None, AL.add)
                lse4 = pe2.tile([128, 4], F32, name="lse4")
                nc.scalar.activation(lse4[:], Zm[:], AF.Ln)

                S4 = pe2.tile([128, 4], F32, name="S4")
                Md4 = pe2.tile([128, 4], F32, name="Md4")
                S2x = pe2.tile([128, 2], F32, name="S2x")
                nc.vector.tensor_scalar_mul(S2x[:], Ssb[:], 2.0)
                for a in range(2):
                    nc.vector.tensor_copy(S4[:, a * 2:a * 2 + 2], S2x[:])
                    nc.vector.tensor_copy(Md4[:, a * 2:a * 2 + 2], Md[:])

                tt = pe2.tile([128, 4], F32, name="tt")
                nc.vector.tensor_tensor(tt[:], lse4[:], negdT[:], AL.subtract)
                m2 = pe2.tile([128, 4], F32, name="m2")
                nc.vector.tensor_tensor(m2[:], tt[:], S4[:], AL.mult)
                m3 = pe2.tile([128, 4], F32, name="m3")
                nc.vector.tensor_tensor(m3[:], Dacc4[:], m2[:], AL.subtract)
                m4 = pe2.tile([128, 4], F32, name="m4")
                nc.vector.tensor_tensor(m4[:], Md4[:], lse4[:], AL.mult)
                mlpp = pe2.tile([128, 4], F32, name="mlpp")
                nc.vector.tensor_tensor(mlpp[:], m3[:], m4[:], AL.add)

                psL = ps_sm.tile([1, 4], F32, name="psL", tag="sm")
                nc.tensor.matmul(psL[:], ones_col[:], mlpp[:], start=True, stop=True)
                Lrow = pe2.tile([1, 4], F32, name="Lrow")
                nc.scalar.copy(Lrow[:], psL[:])
                pl = pe2.tile([1, 1], F32, name="pl")
                nc.vector.reduce_sum(pl[:], Lrow[:], axis=AX.X)
                nc.sync.dma_start(pl_in[:], pl[:])
                if collectives:
                    nc.gpsimd.collective_compute(
                        "AllGather", AL.bypass,
                        replica_groups=[list(range(NCORE))],
                        ins=[pl_in[:]], outs=[pl_out[:]],
                    )
                else:
                    for rr in range(NCORE):
                        nc.sync.dma_start(pl_out[rr:rr + 1], pl_in[:])
                plr = pe2.tile([1, NCORE], F32, name="plr")
                nc.sync.dma_start(plr[:], pl_out[:])
                ptot = pe2.tile([1, 1], F32, name="ptot")
                nc.vector.reduce_sum(ptot[:], plr[:], axis=AX.X)
                lossv = pe2.tile([1, 1], F32, name="lossv")
                nc.vector.tensor_scalar_mul(lossv[:], ptot[:], -1.0 / N)
                nc.sync.dma_start(loss_out[:], lossv[:])

    nc.compile()
    return nc


_CACHE = {}


def get_nc():
    if "nc" not in _CACHE:
        _CACHE["nc"] = build_nc()
    return _CACHE["nc"]


def make_in_maps(outputs, features, Y, predicted_score):
    outputs = np.ascontiguousarray(outputs, dtype=np.float32)
    features = np.ascontiguousarray(features, dtype=np.float32)
    Y = np.ascontiguousarray(Y, dtype=np.float32)
    predicted_score = np.ascontiguousarray(predicted_score, dtype=np.float32)
    F2 = features.reshape(B, V * D)
    iota = np.tile(np.arange(CP, dtype=np.float32), (128, 1))
    ci8 = (np.arange(8, dtype=np.float32)[None, :] * 128
           + np.arange(128, dtype=np.float32)[:, None])
    ident = np.eye(128, dtype=np.float32)
    in_maps = []
    for m in range(NCORE):
        sl = slice(m * BL, (m + 1) * BL)
        in_maps.append({
            "f_full": F2,
            "f_loc": np.ascontiguousarray(F2[sl]),
            "o_loc": np.ascontiguousarray(outputs[sl]),
            "y_loc": np.ascontiguousarray(Y[sl]),
            "ps_loc": np.ascontiguousarray(predicted_score[sl]),
            "iota_in": iota,
            "ci8_in": ci8,
            "ident_in": ident,
        })
    return in_maps


def kernel(outputs, features, Y, predicted_score):
    nc = get_nc()
    in_maps = make_in_maps(outputs, features, Y, predicted_score)
    res = bass_utils.run_bass_kernel_spmd(
        nc, in_maps, core_ids=list(range(NCORE)), trace=False)
    nt = np.concatenate([res.results[m]["nt_out"] for m in range(NCORE)], axis=0)
    loss = np.float32(res.results[0]["loss_out"][0, 0])
    return loss, nt


if __name__ == "__main__":
    nc = get_nc()
    print("compiled OK")
